# revision 9
# baseline (speedup 1.0000x reference)
"""Trainium2 Bass kernel for nn_GaussianLayer (segment_reduce).

Computes ll[b, r, k] = -0.5 * sum_d((x[b, regions[r,d]] - means[r,k,d]) / scales[r,k,d])^2
                       - sum_d log(scales[r,k,d]) - 0.5 * D * log(2*pi)

Strategy v4 (data-parallel over batch across 8 cores, 512 rows each):
  Quadratic-in-x form:  ll[b,(r,k)] = sum_d wsq[r,k,d]*xg[b,r,d]^2
                                     + sum_d wraw[r,k,d]*xg[b,r,d] + const[r,k]
  with xg[b,r,d] = x[b, regions[r,d]], wsq = -0.5/s^2, wraw = m/s^2.

  Host prep does the gather + transpose + squaring + bf16 cast.  Per core the
  device sees one [128, 16*512] SBUF tensor X: 16 column-blocks, one per
  group of 4 regions; partitions 0-63 hold xg rows (p = 16j+d for region-
  local j, dim d), partitions 64-127 hold xg^2 rows.  Weights are 16 static
  block-diagonal [128,128] bf16 blocks matching that row order.  const is
  added on host after the run.

  Device per core is a pure stream:
    - 16 matmuls outT[128 cols, 512 batch] = wt_blk^T @ X_blk (weight-
      stationary; one LDWEIGHTS + one N=512 matmul per block)
    - PSUM -> SBUF drains with f32->bf16 cast, alternating DVE / ACT
    - inputs ride the two low-latency HWDGE rings (sync/scalar) as four
      contiguous 512KB DMAs with 16KB-per-partition runs; outputs mostly
      ride the gpsimd SWDGE ring whose launch latency hides behind compute
  Output is the transposed [2048, 512] bf16 per core; host transposes,
  upcasts, and adds const.
"""

import os
import sys

for _p in ("/opt/trn_rl_repo", "/root/.axon_site/_ro/trn_rl_repo"):
    if os.path.isdir(_p) and _p not in sys.path:
        sys.path.insert(0, _p)

import numpy as np
import ml_dtypes

import concourse.bass as bass
import concourse.tile as tile
from concourse import bacc, mybir
from concourse.bass_utils import run_bass_kernel_spmd

LOG_2PI = 1.8378770664093453
B, F = 4096, 1024
R, K, D = 64, 32, 16
NCORES = 8
BL = B // NCORES      # 512 batch rows per core
NBLK = 16             # blocks of 4 regions: 128 contract rows / 128 out cols
RKCOLS = R * K        # 2048 output columns
N_WARM = 12           # warm-up matmuls to lift PE off the clock-gated p-state

_module_cache = {}


def _build_module():
    if "nc" in _module_cache:
        return _module_cache["nc"]

    nc = bacc.Bacc(
        trn_type="TRN2",
        target_bir_lowering=False,
        debug=False,
        enable_asserts=False,
    )
    bf16 = mybir.dt.bfloat16
    f32 = mybir.dt.float32

    # xi rows 0-63: x chunk0 (blocks 0-7, [64, 8, 512]), 64-127: x chunk1,
    # 128-191: x^2 chunk0, 192-255: x^2 chunk1 — each slab 512KB contiguous
    xi_d = nc.dram_tensor("xi", [256, 8 * BL], bf16, kind="ExternalInput").ap()
    wt_d = nc.dram_tensor("wt", [256, 8 * 128], bf16, kind="ExternalInput").ap()
    # out chunk g (drain-group) = rows 128g:128g+128, [128, 1024] contiguous
    o_d = nc.dram_tensor("o", [8 * 128, 2 * BL], bf16, kind="ExternalOutput").ap()

    with tile.TileContext(nc) as tc:
        with (
            tc.tile_pool(name="persist", bufs=1) as persist,
            tc.tile_pool(name="ps", bufs=3, space="PSUM") as pspool,
            tc.tile_pool(name="wps", bufs=1, space="PSUM") as wpspool,
            tc.tile_pool(name="osb", bufs=1) as opool,
        ):
            wt_t = persist.tile([128, NBLK * 128], bf16, tag="wt")
            xt = persist.tile([128, NBLK * BL], bf16, tag="xt")
            warm = persist.tile([128, 512], bf16, tag="warm")
            nc.vector.memset(warm[:], 0)

            # weights first on each HWDGE ring, then the 512KB x slabs
            nc.sync.dma_start(wt_t[:, 0:1024], wt_d[0:128, :])
            nc.scalar.dma_start(wt_t[:, 1024:2048], wt_d[128:256, :])
            nc.sync.dma_start(xt[0:64, 0:4096], xi_d[0:64, :])
            nc.scalar.dma_start(xt[64:128, 0:4096], xi_d[128:192, :])
            nc.sync.dma_start(xt[0:64, 4096:8192], xi_d[64:128, :])
            nc.scalar.dma_start(xt[64:128, 4096:8192], xi_d[192:256, :])

            wps = wpspool.tile([128, 512], f32)
            for _ in range(N_WARM):
                nc.tensor.matmul(
                    wps[:, 0:256], warm[:, 0:128], warm[:, 0:256],
                    start=True, stop=True,
                )

            for g in range(8):       # drain-group = 2 blocks
                ps = pspool.tile([128, 2 * BL], f32)
                for h in range(2):
                    q = 2 * g + h
                    nc.tensor.matmul(
                        ps[:, BL * h:BL * (h + 1)],
                        wt_t[:, 128 * q:128 * (q + 1)],
                        xt[:, BL * q:BL * (q + 1)],
                        start=True, stop=True,
                    )
                ob = opool.tile([128, 2 * BL], bf16, tag=f"ob{g}")
                if g % 2 == 0:
                    nc.vector.tensor_copy(ob[:], ps[:])
                else:
                    nc.scalar.copy(ob[:], ps[:])
                # early chunks on the gpsimd ring (its SWDGE latency hides
                # behind compute), tail chunks on the low-latency HWDGE rings
                if g < 4:
                    eng = nc.gpsimd
                elif g % 2 == 0:
                    eng = nc.sync
                else:
                    eng = nc.scalar
                eng.dma_start(o_d[128 * g:128 * (g + 1), :], ob[:])

    nc.compile()
    _module_cache["nc"] = nc
    return nc


def _prep_params(regions, means, scales):
    """Fold [R,K,D] params into 16 block-diagonal [128,128] weight blocks."""
    means = np.asarray(means, dtype=np.float64)
    scales = np.asarray(scales, dtype=np.float64)

    inv2 = 1.0 / scales**2                                   # [R,K,D]
    wsq_c = -0.5 * inv2                                      # coeff of x^2
    wraw_c = means * inv2                                    # coeff of x
    const = (
        -0.5 * np.sum(means**2 * inv2, axis=-1)
        - np.sum(np.log(scales), axis=-1)
        - 0.5 * D * LOG_2PI
    ).astype(np.float32)                                     # [R,K]

    # rows 16j+d -> wraw, rows 64+16j+d -> wsq; cols 128q + 32j + k
    wt = np.zeros((128, NBLK * 128), np.float32)
    for q in range(NBLK):
        for j in range(4):
            r = 4 * q + j
            cols = slice(128 * q + 32 * j, 128 * q + 32 * j + 32)
            wt[16 * j:16 * j + 16, cols] = wraw_c[r].T.astype(np.float32)
            wt[64 + 16 * j:64 + 16 * j + 16, cols] = wsq_c[r].T.astype(np.float32)
    # chunk-major: [256, 1024], rows 0:128 = blocks 0-7, 128:256 = blocks 8-15
    wt = np.ascontiguousarray(
        wt.reshape(128, 2, 1024).transpose(1, 0, 2).reshape(256, 1024))
    return wt.astype(ml_dtypes.bfloat16), const


def _prep_x(x, regions):
    """Gather + transpose + square x into per-core [256, 4096] slabs."""
    regions = np.asarray(regions).astype(np.int64)
    xg = np.asarray(x, dtype=np.float32)[:, regions.reshape(-1)]   # [B, 1024]
    xg2 = xg * xg
    xis = []
    for c in range(NCORES):
        sl = slice(c * BL, (c + 1) * BL)
        xi = np.empty((4, 64, 8, BL), np.float32)
        # feature g = 64q + p (p = 16j+d); chunk c2 = blocks 8c2..8c2+7
        xv = xg[sl].T.reshape(NBLK, 64, BL)      # [q, p, b]
        sv = xg2[sl].T.reshape(NBLK, 64, BL)
        xi[0] = xv[0:8].transpose(1, 0, 2)       # x chunk0   [p, qloc, b]
        xi[1] = xv[8:16].transpose(1, 0, 2)      # x chunk1
        xi[2] = sv[0:8].transpose(1, 0, 2)       # x^2 chunk0
        xi[3] = sv[8:16].transpose(1, 0, 2)      # x^2 chunk1
        xis.append(np.ascontiguousarray(
            xi.reshape(256, 8 * BL)).astype(ml_dtypes.bfloat16))
    return xis


def _run(inputs, trace=False, **kwargs):
    wt, const = _prep_params(inputs["regions"], inputs["means"],
                             inputs["scales"])
    xis = _prep_x(inputs["x"], inputs["regions"])

    nc = _build_module()
    in_maps = [{"xi": xis[c], "wt": wt} for c in range(NCORES)]
    res = run_bass_kernel_spmd(
        nc, in_maps, core_ids=list(range(NCORES)), trace=trace, **kwargs
    )

    parts = []
    for c in range(NCORES):
        o = np.asarray(res.results[c]["o"]).astype(np.float32)
        # [8 g, 128 m, 2 h, 512 b] -> logical [m, q, b], q = 2g + h
        o = o.reshape(8, 128, 2, BL).transpose(1, 0, 2, 3)
        # o[32j+k, (q, b)] -> [b, q, j, k] with r = 4q + j
        ll = o.reshape(4, 32, NBLK, BL).transpose(3, 2, 0, 1).reshape(BL, R, K)
        parts.append(ll)
    out = np.concatenate(parts, axis=0) + const[None, :, :]
    return out, res


def kernel(**inputs):
    out, _ = _run(inputs, trace=False)
    return out


# revision 11
# speedup vs baseline: 1.1399x; 1.1399x over previous
"""Trainium2 Bass kernel for nn_GaussianLayer (segment_reduce).

Computes ll[b, r, k] = -0.5 * sum_d((x[b, regions[r,d]] - means[r,k,d]) / scales[r,k,d])^2
                       - sum_d log(scales[r,k,d]) - 0.5 * D * log(2*pi)

Strategy v5 (data-parallel over batch across 8 cores, 512 rows each):
  Quadratic-in-x form:  ll[b,(r,k)] = sum_d wsq[r,k,d]*xg[b,r,d]^2
                                     + sum_d wraw[r,k,d]*xg[b,r,d] + const[r,k]
  with xg[b,r,d] = x[b, regions[r,d]], wsq = -0.5/s^2, wraw = m/s^2.

  Host prep does the gather + transpose + squaring + bf16 cast, packing per
  core a logical [128, 16*512] tensor xi: 16 column-blocks, one per group of
  4 regions; partition p = 32j+16s+d holds (s=0) xg or (s=1) xg^2 rows for
  region-local j, dim d.  Weights are 16 static block-diagonal [128,128]
  bf16 blocks matching that row order.  const is added on host post-run.

  Device per core is a pure stream:
    - 16 matmuls outT[128 cols, 512 batch] = wt_blk^T @ xi_blk (weight-
      stationary; one LDWEIGHTS + one N=512 matmul per block)
    - PSUM -> SBUF drains with f32->bf16 cast, alternating DVE / ACT
    - inputs ride the two low-latency HWDGE rings (sync/scalar) as
      chunk-contiguous 512KB DMAs; outputs ride the gpsimd SWDGE ring
      early (its launch latency hides behind compute) and HWDGE late
  Output is the transposed [2048, 512] bf16 per core; host transposes,
  upcasts, and adds const.
"""

import os
import sys

for _p in ("/opt/trn_rl_repo", "/root/.axon_site/_ro/trn_rl_repo"):
    if os.path.isdir(_p) and _p not in sys.path:
        sys.path.insert(0, _p)

import numpy as np
import ml_dtypes

import concourse.bass as bass
import concourse.tile as tile
from concourse import bacc, mybir
from concourse.bass_utils import run_bass_kernel_spmd

LOG_2PI = 1.8378770664093453
B, F = 4096, 1024
R, K, D = 64, 32, 16
NCORES = 8
BL = B // NCORES      # 512 batch rows per core
NBLK = 16             # blocks of 4 regions: 128 contract rows / 128 out cols
RKCOLS = R * K        # 2048 output columns
N_WARM = 12           # warm-up matmuls to lift PE off the clock-gated p-state

_module_cache = {}


def _build_module():
    if "nc" in _module_cache:
        return _module_cache["nc"]

    nc = bacc.Bacc(
        trn_type="TRN2",
        target_bir_lowering=False,
        debug=False,
        enable_asserts=False,
    )
    bf16 = mybir.dt.bfloat16
    f32 = mybir.dt.float32

    # chunk-major layouts: every row-block below is a contiguous HBM region
    xi_d = nc.dram_tensor("xi", [4 * 128, 4 * BL], bf16, kind="ExternalInput").ap()
    wt_d = nc.dram_tensor("wt", [2 * 128, 8 * 128], bf16, kind="ExternalInput").ap()
    o_d = nc.dram_tensor("o", [8 * 128, 2 * BL], bf16, kind="ExternalOutput").ap()

    with tile.TileContext(nc) as tc:
        with (
            tc.tile_pool(name="persist", bufs=1) as persist,
            tc.tile_pool(name="ps", bufs=4, space="PSUM") as pspool,
            tc.tile_pool(name="osb", bufs=1) as opool,
        ):
            wt_t = persist.tile([128, NBLK * 128], bf16, tag="wt")
            warm = persist.tile([128, 512], bf16, tag="warm")
            nc.vector.memset(warm[:], 0)

            # inputs only on the two HWDGE rings: weights first, then xi
            nc.sync.dma_start(wt_t[:, 0:1024], wt_d[0:128, :])
            nc.scalar.dma_start(wt_t[:, 1024:2048], wt_d[128:256, :])
            xts = []
            for i in range(4):
                xt = persist.tile([128, 4 * BL], bf16, tag=f"xi{i}")
                eng = nc.sync if i % 2 == 0 else nc.scalar
                eng.dma_start(xt[:], xi_d[128 * i:128 * (i + 1), :])
                xts.append(xt)

            # warm-up matmuls share the PSUM rotation (keeps all 8 banks
            # for the real pipeline); they chain on PE program order only
            for _ in range(N_WARM):
                psw = pspool.tile([128, 2 * BL], f32, tag="ps")
                nc.tensor.matmul(
                    psw[:, 0:256], warm[:, 0:128], warm[:, 0:256],
                    start=True, stop=True,
                )

            for g in range(8):       # drain-group = 2 blocks
                ps = pspool.tile([128, 2 * BL], f32, tag="ps")
                for h in range(2):
                    q = 2 * g + h
                    nc.tensor.matmul(
                        ps[:, BL * h:BL * (h + 1)],
                        wt_t[:, 128 * q:128 * (q + 1)],
                        xts[q // 4][:, BL * (q % 4):BL * (q % 4 + 1)],
                        start=True, stop=True,
                    )
                ob = opool.tile([128, 2 * BL], bf16, tag=f"ob{g}")
                if g % 2 == 0:
                    nc.vector.tensor_copy(ob[:], ps[:])
                else:
                    nc.scalar.copy(ob[:], ps[:])
                # early chunks on the gpsimd ring (SWDGE latency hides behind
                # compute), tail chunks on the low-latency HWDGE rings
                if g < 4:
                    eng = nc.gpsimd
                elif g % 2 == 0:
                    eng = nc.sync
                else:
                    eng = nc.scalar
                eng.dma_start(o_d[128 * g:128 * (g + 1), :], ob[:])

    nc.compile()
    _module_cache["nc"] = nc
    return nc


def _prep_params(regions, means, scales):
    """Fold [R,K,D] params into 16 block-diagonal [128,128] weight blocks."""
    means = np.asarray(means, dtype=np.float64)
    scales = np.asarray(scales, dtype=np.float64)

    inv2 = 1.0 / scales**2                                   # [R,K,D]
    wsq_c = -0.5 * inv2                                      # coeff of x^2
    wraw_c = means * inv2                                    # coeff of x
    const = (
        -0.5 * np.sum(means**2 * inv2, axis=-1)
        - np.sum(np.log(scales), axis=-1)
        - 0.5 * D * LOG_2PI
    ).astype(np.float32)                                     # [R,K]

    # wt[32j+16s+d, 128q + 32j + k]: s=0 -> wraw, s=1 -> wsq for region 4q+j
    wt = np.zeros((128, NBLK * 128), np.float32)
    for q in range(NBLK):
        for j in range(4):
            r = 4 * q + j
            cols = slice(128 * q + 32 * j, 128 * q + 32 * j + 32)
            wt[32 * j:32 * j + 16, cols] = wraw_c[r].T.astype(np.float32)
            wt[32 * j + 16:32 * j + 32, cols] = wsq_c[r].T.astype(np.float32)
    # chunk-major: [256, 1024], rows 0:128 = blocks 0-7, 128:256 = blocks 8-15
    wt = np.ascontiguousarray(
        wt.reshape(128, 2, 1024).transpose(1, 0, 2).reshape(256, 1024))
    return wt.astype(ml_dtypes.bfloat16), const


def _prep_x(x, regions):
    """Gather + transpose + square + interleave x into per-core xi tensors."""
    regions = np.asarray(regions).astype(np.int64)
    xg = np.asarray(x, dtype=np.float32)[:, regions.reshape(-1)]   # [B, 1024]
    xg2 = xg * xg
    xis = []
    for c in range(NCORES):
        sl = slice(c * BL, (c + 1) * BL)
        xi = np.empty((4, 2, 16, NBLK, BL), np.float32)
        # feature g = 64q + 16j + d  ->  reshape (q, j, d) on the T side
        xi[:, 0] = xg[sl].T.reshape(NBLK, 4, 16, BL).transpose(1, 2, 0, 3)
        xi[:, 1] = xg2[sl].T.reshape(NBLK, 4, 16, BL).transpose(1, 2, 0, 3)
        # chunk-major: [512, 2048], row-block c2 = blocks 4*c2..4*c2+3
        xic = xi.reshape(128, 4, 4 * BL).transpose(1, 0, 2).reshape(512, 4 * BL)
        xis.append(np.ascontiguousarray(xic).astype(ml_dtypes.bfloat16))
    return xis


def _run(inputs, trace=False, **kwargs):
    wt, const = _prep_params(inputs["regions"], inputs["means"],
                             inputs["scales"])
    xis = _prep_x(inputs["x"], inputs["regions"])

    nc = _build_module()
    in_maps = [{"xi": xis[c], "wt": wt} for c in range(NCORES)]
    res = run_bass_kernel_spmd(
        nc, in_maps, core_ids=list(range(NCORES)), trace=trace, **kwargs
    )

    parts = []
    for c in range(NCORES):
        o = np.asarray(res.results[c]["o"]).astype(np.float32)
        # [8 g, 128 m, 2 h, 512 b] -> logical [m, q, b], q = 2g + h
        o = o.reshape(8, 128, 2, BL).transpose(1, 0, 2, 3)
        # o[32j+k, (q, b)] -> [b, q, j, k] with r = 4q + j
        ll = o.reshape(4, 32, NBLK, BL).transpose(3, 2, 0, 1).reshape(BL, R, K)
        parts.append(ll)
    out = np.concatenate(parts, axis=0) + const[None, :, :]
    return out, res


def kernel(**inputs):
    out, _ = _run(inputs, trace=False)
    return out
